# revision 1
# baseline (speedup 1.0000x reference)
"""Trainium2 Bass kernel for an AttnBlock (GroupNorm -> single-head attention
-> out-proj -> residual) on x[2, 512, 64, 64].

Sharding: 8 cores = batch(2) x query-chunk(4). Each core receives its batch's
full x with its own 1024 query columns permuted to the front (GroupNorm stats
and softmax sums over spatial positions are permutation invariant), computes
GN + V for all 4096 positions, and attention + out-proj for its 1024 queries.

Algebra: scores[j,i] = (q_i+bq).(k_j+bk) = hn_i^T M hn_j + t[j] + (terms
constant in j, which softmax ignores), with M = wq^T wk * c^-0.5 and
t[j] = (bq^T wk * c^-0.5) . hn_j.  This removes the K GEMM entirely.
All GEMMs run in bf16 with fp32 PSUM accumulation.  The program is
specialized host-side on whether bq / bv are all-zero (they are for the
reference inputs), which drops the t / bias matmuls.
"""

import numpy as np
import ml_dtypes

import concourse.bass as bass
import concourse.tile as tile
from concourse import mybir

P = 128
C = 512
N = 4096
NQ = 1024          # queries per core
CCN = 4            # channel chunks of 128
NB = 8             # n chunks of 512
JCN = 32           # j chunks of 128
IBN = 2            # i blocks of 512 per core
SCALE = float(C) ** -0.5
EPS = 1e-6
GROUP = 16         # channels per group

F32 = mybir.dt.float32
BF16 = mybir.dt.bfloat16
AF = mybir.ActivationFunctionType
ALU = mybir.AluOpType
BF16NP = ml_dtypes.bfloat16

_WAIT_LIMIT = 1


def _split_excess_waits(nc):
    """This walrus build rejects multi-wait sync on one instruction.  Move
    excess waits onto same-engine NoOps inserted just before the offending
    instruction; engine queues (and the SP DMA-trigger stream) are FIFO, so
    semantics are preserved."""
    counter = 0
    for f in nc.m.functions:
        for bb in f.blocks:
            insts = bb.instructions
            out = []
            for ins in insts:
                si = ins.sync_info
                waits = list(si.on_wait) if si and si.on_wait else []
                if len(waits) > _WAIT_LIMIT:
                    si.on_wait = waits[-_WAIT_LIMIT:]
                    extra = waits[:-_WAIT_LIMIT]
                    for i in range(0, len(extra), _WAIT_LIMIT):
                        nop = mybir.InstNoOp(
                            name=f"I-wsplit-{counter}", ins=[], outs=[])
                        counter += 1
                        nop.engine = ins.engine
                        nop.sync_info = mybir.SyncInfo(
                            on_wait=extra[i:i + _WAIT_LIMIT], on_update=[])
                        out.append(nop)
                out.append(ins)
            insts[:] = out


def build_program(with_t=True, with_bv=True, with_bo=True, split_waits=True):
    nc = bass.Bass("TRN2", target_bir_lowering=False, debug=False)

    xp = nc.dram_tensor("xp", [C, N], F32, kind="ExternalInput").ap()
    wq_d = nc.dram_tensor("wq", [C, C], F32, kind="ExternalInput").ap()
    wk_d = nc.dram_tensor("wk", [C, C], F32, kind="ExternalInput").ap()
    wv_d = nc.dram_tensor("wv", [C, C], F32, kind="ExternalInput").ap()
    wo_d = nc.dram_tensor("wo", [C, C], F32, kind="ExternalInput").ap()
    bq_d = nc.dram_tensor("bq", [C], F32, kind="ExternalInput").ap()
    bv_d = nc.dram_tensor("bv", [C], F32, kind="ExternalInput").ap()
    bo_d = nc.dram_tensor("bo", [C], F32, kind="ExternalInput").ap()
    gam_d = nc.dram_tensor("gamma", [C], F32, kind="ExternalInput").ap()
    bet_d = nc.dram_tensor("beta", [C], F32, kind="ExternalInput").ap()
    sel_d = nc.dram_tensor("sel", [P, 8], F32, kind="ExternalInput").ap()
    bsel_d = nc.dram_tensor("bsel", [8, P], F32, kind="ExternalInput").ap()
    id_d = nc.dram_tensor("ident", [P, P], BF16, kind="ExternalInput").ap()
    ones_d = nc.dram_tensor("onesb", [P, C], BF16, kind="ExternalInput").ap()
    onesf_d = nc.dram_tensor("onesf", [1, P], F32, kind="ExternalInput").ap()
    out_d = nc.dram_tensor("out", [C, NQ], F32, kind="ExternalOutput").ap()

    xv = xp.rearrange("(cc p) n -> p cc n", p=P)
    ov = out_d.rearrange("(oc p) n -> p oc n", p=P)

    with tile.TileContext(nc) as tc:
        _emit(nc, tc, xv, ov,
              dict(wq=wq_d, wk=wk_d, wv=wv_d, wo=wo_d),
              dict(bq=bq_d, bv=bv_d, bo=bo_d, gam=gam_d, bet=bet_d),
              dict(sel=sel_d, bsel=bsel_d, ident=id_d, onesb=ones_d,
                   onesf=onesf_d),
              with_t=with_t, with_bv=with_bv, with_bo=with_bo)
    if split_waits:
        _split_excess_waits(nc)
    return nc


def _emit(nc, tc, xv, ov, wd, vd, cd, with_t, with_bv, with_bo):
    from contextlib import ExitStack
    ctx = ExitStack()
    with ctx:
        const = ctx.enter_context(tc.tile_pool(name="const", bufs=1))
        persist = ctx.enter_context(tc.tile_pool(name="persist", bufs=1))
        evac = ctx.enter_context(tc.tile_pool(name="evac", bufs=2))
        dram = ctx.enter_context(tc.tile_pool(name="dram", bufs=1, space="DRAM"))

        # ---- constants / small vectors ----
        sel = const.tile([P, 8], F32)
        nc.sync.dma_start(sel[:], cd["sel"][:])
        bsel = const.tile([8, P], F32)
        nc.sync.dma_start(bsel[:], cd["bsel"][:])
        ident = const.tile([P, P], BF16)
        nc.sync.dma_start(ident[:], cd["ident"][:])
        onesb = const.tile([P, C], BF16)
        nc.sync.dma_start(onesb[:], cd["onesb"][:])
        onesf = const.tile([1, P], F32)
        nc.sync.dma_start(onesf[:], cd["onesf"][:])

        def vec128(name, src):
            t = const.tile([P, CCN], F32, name=name)
            nc.sync.dma_start(t[:], src.rearrange("(cc p) -> p cc", p=P))
            return t

        bq_sb = vec128("bq_sb", vd["bq"])
        bo_sb = vec128("bo_sb", vd["bo"])
        gam_sb = vec128("gam_sb", vd["gam"])
        bet_sb = vec128("bet_sb", vd["bet"])
        bv_bf = const.tile([1, C], BF16)
        if with_bv:
            bv_sb = const.tile([1, C], F32)
            nc.sync.dma_start(bv_sb[:], vd["bv"].rearrange("(a n) -> a n", a=1))
            nc.vector.tensor_copy(bv_bf[:], bv_sb[:])
        bq_bf = const.tile([P, CCN], BF16)
        nc.vector.tensor_copy(bq_bf[:], bq_sb[:])

        M_bf = persist.tile([P, CCN, C], BF16)     # M[c, c'] * scale, c=cc*128+p
        wvT_bf = persist.tile([P, CCN, C], BF16)   # wv^T[i, o], i=ic*128+p
        woT_bf = persist.tile([P, CCN, C], BF16)   # wo^T[c, o]
        ut_bf = const.tile([P, CCN], BF16)         # u[c'] * scale
        t_part = const.tile([P, JCN], F32)         # t[j] laid out [p, jc]
        hn = persist.tile([P, CCN, N], BF16)
        q2_sb = persist.tile([P, CCN, NQ], BF16)
        vT = persist.tile([P, JCN, C], BF16)
        A_sb = const.tile([P, CCN], F32)
        B_sb = const.tile([P, CCN], F32)
        bnbuf = const.tile([P, CCN, NB, 6], F32)
        mv = const.tile([P, CCN, 2], F32)

        pearly = tc.alloc_tile_pool(name="pearly", bufs=3, space="PSUM")

        # ---- single-pass x load (resident) + GN stats ----
        # The 16 HWDGE queues share HBM bandwidth roughly equally, so a
        # monolithic 1MB tile DMA would finish only when ALL queued traffic
        # finishes (~40us).  Split every tile into 8 pieces spread over the
        # queues so tile 0 (and the wq/wk quarters interleaved below) land
        # in a few microseconds and the pipeline starts early.
        xpool = tc.alloc_tile_pool(name="xres", bufs=1)

        # ---- weight staging + bf16 convert + transposes + M/uT ----
        with tc.tile_pool(name="wload", bufs=2) as wload:
            def load_bf(name, d):
                dv = d.rearrange("(oc p) i -> p oc i", p=P)
                bf = wload.tile([P, CCN, C], BF16, name=f"{name}_bf", tag="bf")
                for h in range(CCN):
                    st = wload.tile([P, 1, C], F32, name=f"{name}_st{h}", tag="st")
                    nc.sync.dma_start(st[:], dv[:, h:h + 1, :])
                    nc.scalar.copy(bf[:, h:h + 1, :], st[:])
                return bf

            # wq/wk quarters first (2MB; M unblocks early), then x as 16
            # 1MB pieces with 8KB contiguous lines filling every DMA queue
            # at full bandwidth; wv/wo follow behind x.
            wq_bf = load_bf("wq", wd["wq"])
            wk_bf = load_bf("wk", wd["wk"])
            xfull = xpool.tile([P, CCN, N], F32, name="xfull", tag="x")
            # Non-uniform pieces: small leading chunks land fast (the stats
            # pipeline starts), large trailing chunks keep 4-8KB DMA lines.
            # wv/wo are queued between the leading and trailing x pieces so
            # their transposes are ready before vT needs them.
            for h in range(2):
                for cc in range(CCN):
                    nc.sync.dma_start(
                        xfull[:, cc, h * 512:(h + 1) * 512],
                        xv[:, cc, h * 512:(h + 1) * 512])
            wv_bf = load_bf("wv", wd["wv"])
            wo_bf = load_bf("wo", wd["wo"])
            bounds = [1024, 2048, N]
            for h in range(len(bounds) - 1):
                for cc in range(CCN):
                    nc.sync.dma_start(
                        xfull[:, cc, bounds[h]:bounds[h + 1]],
                        xv[:, cc, bounds[h]:bounds[h + 1]])
            for nb in range(NB):
                for cc in range(CCN):
                    nc.vector.bn_stats(bnbuf[:, cc, nb, :],
                                       xfull[:, cc, nb * 512:(nb + 1) * 512])

            # M = wq^T wk * scale
            for cch in range(CCN):
                m_ps = pearly.tile([P, C], F32, name="m_ps", tag="big")
                for oc in range(CCN):
                    nc.tensor.matmul(m_ps[:], wq_bf[:, oc, cch * P:(cch + 1) * P],
                                     wk_bf[:, oc, :],
                                     start=(oc == 0), stop=(oc == CCN - 1))
                if cch % 2 == 0:
                    nc.scalar.mul(M_bf[:, cch, :], m_ps[:], SCALE)
                else:
                    nc.vector.tensor_scalar_mul(M_bf[:, cch, :], m_ps[:], SCALE)

            if with_t:
                # uT[c'] = sum_o wk[o, c'] bq[o]   (scaled)
                ut_ps = pearly.tile([P, CCN], F32, tag="big")
                for cch in range(CCN):
                    for oc in range(CCN):
                        nc.tensor.matmul(ut_ps[:, cch:cch + 1],
                                         wk_bf[:, oc, cch * P:(cch + 1) * P],
                                         bq_bf[:, oc:oc + 1],
                                         start=(oc == 0), stop=(oc == CCN - 1),
                                         skip_group_check=True)
                nc.scalar.mul(ut_bf[:], ut_ps[:], SCALE)

            # transposes of wv, wo via PE
            for w_bf, dst in ((wv_bf, wvT_bf), (wo_bf, woT_bf)):
                for oc in range(CCN):
                    for ic in range(CCN):
                        tp_ps = pearly.tile([P, P], BF16, name="tp_ps", tag="big")
                        nc.tensor.transpose(tp_ps[:], w_bf[:, oc, ic * P:(ic + 1) * P],
                                            ident[:])
                        if (oc * CCN + ic) % 2 == 0:
                            nc.scalar.copy(dst[:, ic, oc * P:(oc + 1) * P], tp_ps[:])
                        else:
                            nc.vector.tensor_copy(dst[:, ic, oc * P:(oc + 1) * P],
                                                  tp_ps[:])

        # ---- GN stat aggregation -> per-channel A, B ----
        for cc in range(CCN):
            nc.vector.bn_aggr(mv[:, cc, :],
                              bnbuf[:, cc, :, :].rearrange("p a b -> p (a b)"))
        stats8 = const.tile([P, 8], F32)
        nc.vector.tensor_copy(stats8[:, 0:4], mv[:, :, 0])
        nc.vector.tensor_mul(stats8[:, 4:8], mv[:, :, 0], mv[:, :, 0])
        nc.vector.tensor_add(stats8[:, 4:8], stats8[:, 4:8], mv[:, :, 1])
        gs_ps = pearly.tile([8, 8], F32, tag="big")
        nc.tensor.matmul(gs_ps[:], sel[:], stats8[:], start=True, stop=True)
        gs_sb = const.tile([8, 8], F32)
        nc.vector.tensor_copy(gs_sb[:], gs_ps[:])
        gvar = const.tile([8, 4], F32)
        nc.vector.tensor_mul(gvar[:], gs_sb[:, 0:4], gs_sb[:, 0:4])
        nc.vector.tensor_sub(gvar[:], gs_sb[:, 4:8], gvar[:])
        nc.vector.tensor_scalar_add(gvar[:], gvar[:], EPS)
        gsq = const.tile([8, 4], F32)
        nc.scalar.sqrt(gsq[:], gvar[:])
        grs2 = const.tile([8, 8], F32)
        nc.vector.tensor_copy(grs2[:, 0:4], gs_sb[:, 0:4])
        nc.vector.reciprocal(grs2[:, 4:8], gsq[:])
        bc_ps = pearly.tile([P, 8], F32, tag="big")
        nc.tensor.matmul(bc_ps[:], bsel[:], grs2[:], start=True, stop=True)
        nc.vector.tensor_mul(A_sb[:], gam_sb[:], bc_ps[:, 4:8])
        nc.vector.scalar_tensor_tensor(B_sb[:], bc_ps[:, 0:4], -1.0, A_sb[:],
                                       op0=ALU.mult, op1=ALU.mult)
        nc.vector.tensor_add(B_sb[:], B_sb[:], bet_sb[:])

        # ---- apply GN -> hn (bf16), reading the resident x ----
        for nb in range(NB):
            for cc in range(CCN):
                dst = hn[:, cc, nb * 512:(nb + 1) * 512]
                xsl = xfull[:, cc, nb * 512:(nb + 1) * 512]
                if cc < 2:
                    nc.vector.tensor_scalar(dst, xsl,
                                            A_sb[:, cc:cc + 1],
                                            B_sb[:, cc:cc + 1],
                                            op0=ALU.mult, op1=ALU.add)
                elif cc == 2:
                    nc.scalar.activation(dst, xsl, AF.Identity,
                                         bias=B_sb[:, cc:cc + 1],
                                         scale=A_sb[:, cc:cc + 1])
                else:
                    nc.gpsimd.tensor_scalar(dst, xsl,
                                            A_sb[:, cc:cc + 1],
                                            B_sb[:, cc:cc + 1],
                                            op0=ALU.mult, op1=ALU.add)
        xpool.release()

        # ---- q2[c', i] = sum_c M[c, c'] hn[c, i]  (i in 0:1024) ----
        for cch in range(CCN):
            for ih in range(2):
                q_ps = pearly.tile([P, 512], F32, name="q_ps", tag="big")
                for cc in range(CCN):
                    nc.tensor.matmul(q_ps[:], M_bf[:, cc, cch * P:(cch + 1) * P],
                                     hn[:, cc, ih * 512:(ih + 1) * 512],
                                     start=(cc == 0), stop=(cc == CCN - 1))
                if (cch * 2 + ih) % 2 == 0:
                    nc.scalar.copy(q2_sb[:, cch, ih * 512:(ih + 1) * 512], q_ps[:])
                else:
                    nc.vector.tensor_copy(q2_sb[:, cch, ih * 512:(ih + 1) * 512],
                                          q_ps[:])

        if with_t:
            # t[n] = sum_c' uT[c'] hn[c', n]  -> DRAM bounce -> t_part[p, jc]
            t_dram = dram.tile([N], F32)
            for nb in range(NB):
                t_ps = pearly.tile([1, 512], F32, name="t_ps", tag="big")
                for cch in range(CCN):
                    nc.tensor.matmul(t_ps[:], ut_bf[:, cch:cch + 1],
                                     hn[:, cch, nb * 512:(nb + 1) * 512],
                                     start=(cch == 0), stop=(cch == CCN - 1))
                t_ch = evac.tile([1, 512], F32, name="t_ch", tag="tch", bufs=1)
                nc.scalar.copy(t_ch[:], t_ps[:])
                nc.sync.dma_start(t_dram[nb * 512:(nb + 1) * 512], t_ch[:])
            nc.sync.dma_start(t_part[:], t_dram.rearrange("(jc p) -> p jc", p=P))

        # ---- vT[n, o] = sum_i hn[i, n] wv^T[i, o] (+ bv[o]) ----
        for jc in range(JCN):
            v_ps = pearly.tile([P, C], F32, name="v_ps", tag="big")
            for cc in range(CCN):
                nc.tensor.matmul(v_ps[:], hn[:, cc, jc * P:(jc + 1) * P],
                                 wvT_bf[:, cc, :],
                                 start=(cc == 0),
                                 stop=(cc == CCN - 1 and not with_bv),
                                 skip_group_check=True)
            if with_bv:
                nc.tensor.matmul(v_ps[:], onesb[0:1, 0:P], bv_bf[:],
                                 start=False, stop=True, skip_group_check=True)
            if jc % 2 == 0:
                nc.scalar.copy(vT[:, jc, :], v_ps[:])
            else:
                nc.vector.tensor_copy(vT[:, jc, :], v_ps[:])

        pearly.release()

        # ---- attention ----
        # Emit A(ib0), A(ib1) (scores+exp, with the softmax denominator
        # accumulated on DVE as exp tiles appear), then B(ib0), B(ib1)
        # (AV matmuls).  AV is evacuated UNNORMALIZED; 1/denominator is
        # folded into the out-proj evacuation, which removes the PE stall
        # on the reciprocal chain.
        aTpool = tc.alloc_tile_pool(name="aT", bufs=66)
        patt = tc.alloc_tile_pool(name="patt", bufs=1, space="PSUM")
        aTs = {}
        dacc = [const.tile([P, 512], F32, name=f"dacc{ib}") for ib in range(IBN)]
        for ib in range(IBN):
            i0 = ib * 512
            for jc in range(JCN):
                s_ps = patt.tile([P, 512], F32, name="s_ps", tag="s", bufs=3)
                for cc in range(CCN):
                    nc.tensor.matmul(s_ps[:], hn[:, cc, jc * P:(jc + 1) * P],
                                     q2_sb[:, cc, i0:i0 + 512],
                                     start=(cc == 0), stop=(cc == CCN - 1))
                aT_t = aTpool.tile([P, 512], BF16, name="aT_t", tag="aT", bufs=66)
                if with_t:
                    nc.scalar.activation(aT_t[:], s_ps[:], AF.Exp,
                                         bias=t_part[:, jc:jc + 1], scale=1.0)
                else:
                    nc.scalar.activation(aT_t[:], s_ps[:], AF.Exp)
                aTs[ib, jc] = aT_t
                if jc == 0:
                    nc.vector.tensor_copy(dacc[ib][:], aT_t[:])
                else:
                    nc.vector.tensor_add(dacc[ib][:], dacc[ib][:], aT_t[:])

        bcsts = []
        for ib in range(IBN):
            # partition-sum of dacc -> denominator; reciprocal; broadcast
            daccb = evac.tile([P, 512], BF16, name="daccb", tag="daccb", bufs=1)
            nc.vector.tensor_copy(daccb[:], dacc[ib][:])
            den_ps = patt.tile([1, 512], F32, name="den_ps", tag="s", bufs=3)
            nc.tensor.matmul(den_ps[:], onesb[:, 0:1], daccb[:],
                             start=True, stop=True)
            recip = evac.tile([1, 512], BF16, name="recip", tag="recip")
            with nc.allow_low_precision(reason="bf16 1/denominator is ample"):
                nc.vector.reciprocal(recip[:], den_ps[:])
            bcst_ps = patt.tile([P, 512], F32, name="bcst_ps", tag="s", bufs=3)
            nc.tensor.matmul(bcst_ps[:], onesb[0:1, 0:P], recip[:],
                             start=True, stop=True)
            bcst_sb = const.tile([P, 512], F32, name=f"bcst{ib}")
            nc.scalar.copy(bcst_sb[:], bcst_ps[:])
            bcsts.append(bcst_sb)

            i0 = ib * 512
            av_ps = [patt.tile([P, 512], F32, name=f"av_ps{cc}", tag="av", bufs=5)
                     for cc in range(CCN)]
            for jc in range(JCN):
                for cc in range(CCN):
                    nc.tensor.matmul(av_ps[cc][:], vT[:, jc, cc * P:(cc + 1) * P],
                                     aTs[ib, jc][:],
                                     start=(jc == 0), stop=(jc == JCN - 1),
                                     skip_group_check=True)
            av_sb = evac.tile([P, CCN, 512], BF16, name="av_sb", tag="avsb", bufs=1)
            for cc in range(CCN):
                if cc % 2 == 0:
                    nc.vector.tensor_copy(av_sb[:, cc, :], av_ps[cc][:])
                else:
                    nc.scalar.copy(av_sb[:, cc, :], av_ps[cc][:])
            # out-proj, then normalize + bias + residual during evacuation;
            # the evac chains alternate DVE / GpSimd so the final block's
            # writeback is not serialized behind one engine's backlog.
            for oc in range(CCN):
                op_ps = patt.tile([P, 512], F32, name="op_ps", tag="s", bufs=3)
                for cc in range(CCN):
                    nc.tensor.matmul(op_ps[:], woT_bf[:, cc, oc * P:(oc + 1) * P],
                                     av_sb[:, cc, :],
                                     start=(cc == 0), stop=(cc == CCN - 1))
                xr = evac.tile([P, 512], F32, name="xr", tag="xr")
                nc.sync.dma_start(xr[:], xv[:, oc, i0:i0 + 512])
                osb = evac.tile([P, 512], F32, name="osb", tag="osb")
                nc.vector.tensor_mul(osb[:], op_ps[:], bcst_sb[:])
                if with_bo:
                    nc.vector.scalar_tensor_tensor(osb[:], osb[:],
                                                   bo_sb[:, oc:oc + 1], xr[:],
                                                   op0=ALU.add, op1=ALU.add)
                else:
                    nc.gpsimd.tensor_add(osb[:], osb[:], xr[:])
                nc.sync.dma_start(ov[:, oc, i0:i0 + 512], osb[:])

        aTpool.release()
        patt.release()


# ---------------- host side ----------------

_CACHED = {}


def _get_nc(with_t, with_bv, with_bo=True):
    key = (with_t, with_bv, with_bo)
    if key not in _CACHED:
        _CACHED[key] = build_program(with_t=with_t, with_bv=with_bv,
                                     with_bo=with_bo)
    return _CACHED[key]


def _host_constants():
    p = np.arange(P)
    sel = np.zeros((P, 8), np.float32)
    sel[p, p // GROUP] = 1.0 / GROUP
    bsel = np.zeros((8, P), np.float32)
    bsel[p // GROUP, p] = 1.0
    ident = np.eye(P, dtype=BF16NP)
    onesb = np.ones((P, C), dtype=BF16NP)
    onesf = np.ones((1, P), np.float32)
    return dict(sel=sel, bsel=bsel, ident=ident, onesb=onesb, onesf=onesf)


def kernel(x, gn_scale, gn_bias, wq, bq, wk, bk, wv, bv, wo, bo):
    from concourse.bass_utils import run_bass_kernel_spmd

    bq = np.asarray(bq, np.float32)
    bv = np.asarray(bv, np.float32)
    bo_a = np.asarray(bo, np.float32)
    with_t = bool(np.any(bq != 0))
    with_bv = bool(np.any(bv != 0))
    with_bo = bool(np.any(bo_a != 0))
    nc = _get_nc(with_t, with_bv, with_bo)
    consts = _host_constants()
    xr = np.ascontiguousarray(np.asarray(x, np.float32).reshape(2, C, N))
    shared = dict(
        wq=np.asarray(wq, np.float32), wk=np.asarray(wk, np.float32),
        wv=np.asarray(wv, np.float32), wo=np.asarray(wo, np.float32),
        bq=bq, bv=bv, bo=np.asarray(bo, np.float32),
        gamma=np.asarray(gn_scale, np.float32),
        beta=np.asarray(gn_bias, np.float32),
        **consts,
    )
    in_maps = []
    for core in range(8):
        b, qc = divmod(core, 4)
        perm_x = np.concatenate(
            [xr[b][:, qc * NQ:(qc + 1) * NQ],
             np.delete(xr[b], np.s_[qc * NQ:(qc + 1) * NQ], axis=1)], axis=1)
        in_maps.append({"xp": np.ascontiguousarray(perm_x), **shared})

    res = run_bass_kernel_spmd(nc, in_maps, core_ids=list(range(8)))
    y = np.empty((2, C, N), np.float32)
    for core in range(8):
        b, qc = divmod(core, 4)
        y[b][:, qc * NQ:(qc + 1) * NQ] = res.results[core]["out"]
    return y.reshape(2, C, 64, 64)



# revision 4
# speedup vs baseline: 1.5741x; 1.5741x over previous
"""Trainium2 Bass kernel for an AttnBlock (GroupNorm -> single-head attention
-> out-proj -> residual) on x[2, 512, 64, 64].

Sharding: 8 cores = batch(2) x query-chunk(4).  Each core receives its batch's
full x with its own 1024 query columns permuted to the front (GroupNorm stats
and softmax sums over spatial positions are permutation invariant), computes
GN for all 4096 positions, and attention for its 1024 queries.

Weight algebra is folded HOST-side (weights-only transforms, O(C^2)):
  M   = wq^T wk * c^-0.5      so scores[j,i] = hn_j^T M hn_i (+ t[j] terms)
  W2  = wo wv                 so out = W2 (hn A) / den + b2 + x
  b2  = wo bv + bo
The device computes, all in fp8(e4m3) DoubleRow matmuls with fp32 PSUM:
  q2   = M^T hn               (own 1024 queries)
  P2T  = (W2 hn)^T            [j, o] orientation, one GEMM, no transposes
  s    = hn^T q2 ; a = exp(s) ; den = sum_j a
  out  = (P2T^T a) / den + x  (attention + out-proj fused in ONE GEMM)
"""

import numpy as np
import ml_dtypes

import concourse.bass as bass
import concourse.tile as tile
from concourse import mybir

P = 128
C = 512
N = 4096
NQ = 1024          # queries per core
CCN = 4            # channel chunks of 128
NB = 8             # n chunks of 512
JCN = 32           # j chunks of 128
UCN = 16           # j chunk pairs (DoubleRow)
IBN = 2            # i blocks of 512 per core
SCALE = float(C) ** -0.5
EPS = 1e-6
GROUP = 16         # channels per group

# fp8 scale plan (see module docstring algebra):
SM = 1024.0        # M8 = fp8(M * SM)
SQ = 64.0          # q28 = fp8(q2 * SQ) = fp8(q_psum * SQ/SM)
SW2 = 512.0        # W2T8 = fp8(W2^T * SW2)
SPP = 16.0         # P2T8 = fp8(P2T * SPP) = fp8(p_psum * SPP/SW2)
SU = 64.0          # u8 = fp8(u * SU) for the t-vector path

F32 = mybir.dt.float32
BF16 = mybir.dt.bfloat16
FP8 = mybir.dt.float8e4
AF = mybir.ActivationFunctionType
ALU = mybir.AluOpType
DR = mybir.MatmulPerfMode.DoubleRow
BF16NP = ml_dtypes.bfloat16
FP8NP = ml_dtypes.float8_e4m3

_WAIT_LIMIT = 1


def _split_excess_waits(nc):
    """This walrus build rejects multi-wait sync on one instruction.  Move
    excess waits onto same-engine NoOps inserted just before the offending
    instruction; engine queues (and the SP DMA-trigger stream) are FIFO, so
    semantics are preserved."""
    counter = 0
    for f in nc.m.functions:
        for bb in f.blocks:
            insts = bb.instructions
            out = []
            for ins in insts:
                si = ins.sync_info
                waits = list(si.on_wait) if si and si.on_wait else []
                if len(waits) > _WAIT_LIMIT:
                    si.on_wait = waits[-_WAIT_LIMIT:]
                    extra = waits[:-_WAIT_LIMIT]
                    for i in range(0, len(extra), _WAIT_LIMIT):
                        nop = mybir.InstNoOp(
                            name=f"I-wsplit-{counter}", ins=[], outs=[])
                        counter += 1
                        nop.engine = ins.engine
                        nop.sync_info = mybir.SyncInfo(
                            on_wait=extra[i:i + _WAIT_LIMIT], on_update=[])
                        out.append(nop)
                out.append(ins)
            insts[:] = out


def build_program(with_t=False, with_b2=False, split_waits=True):
    nc = bass.Bass("TRN2", target_bir_lowering=False, debug=False)

    xp = nc.dram_tensor("xp", [C, N], F32, kind="ExternalInput").ap()
    m8_d = nc.dram_tensor("m8", [C, C], FP8, kind="ExternalInput").ap()
    w2t8_d = nc.dram_tensor("w2t8", [C, C], FP8, kind="ExternalInput").ap()
    u8_d = nc.dram_tensor("u8", [C], FP8, kind="ExternalInput").ap()
    b2_d = nc.dram_tensor("b2", [C], F32, kind="ExternalInput").ap()
    gam_d = nc.dram_tensor("gamma", [C], F32, kind="ExternalInput").ap()
    bet_d = nc.dram_tensor("beta", [C], F32, kind="ExternalInput").ap()
    sel_d = nc.dram_tensor("sel", [P, 8], F32, kind="ExternalInput").ap()
    bsel_d = nc.dram_tensor("bsel", [8, P], F32, kind="ExternalInput").ap()
    ones_d = nc.dram_tensor("onesb", [P, 2], BF16, kind="ExternalInput").ap()
    cb_d = nc.dram_tensor("cb16", [2, P], BF16, kind="ExternalInput").ap()
    out_d = nc.dram_tensor("out", [C, NQ], F32, kind="ExternalOutput").ap()

    xv = xp.rearrange("(cc p) n -> p cc n", p=P)
    m8v = m8_d.rearrange("(cc p) o -> p cc o", p=P)
    w2v = w2t8_d.rearrange("(cc p) o -> p cc o", p=P)
    ov = out_d.rearrange("(oc p) n -> p oc n", p=P)

    with tile.TileContext(nc) as tc:
        _emit(nc, tc, xv, ov, m8v, w2v,
              dict(u8=u8_d, b2=b2_d, gam=gam_d, bet=bet_d),
              dict(sel=sel_d, bsel=bsel_d, onesb=ones_d, cb16=cb_d),
              with_t=with_t, with_b2=with_b2)
    if split_waits:
        _split_excess_waits(nc)
    return nc


def _emit(nc, tc, xv, ov, m8v, w2v, vd, cd, with_t, with_b2):
    from contextlib import ExitStack
    ctx = ExitStack()
    with ctx:
        const = ctx.enter_context(tc.tile_pool(name="const", bufs=1))
        persist = ctx.enter_context(tc.tile_pool(name="persist", bufs=1))
        evac = ctx.enter_context(tc.tile_pool(name="evac", bufs=2))
        dram = ctx.enter_context(tc.tile_pool(name="dram", bufs=1, space="DRAM"))

        # ---- constants / small vectors ----
        sel = const.tile([P, 8], F32)
        nc.sync.dma_start(sel[:], cd["sel"][:])
        bsel = const.tile([8, P], F32)
        nc.sync.dma_start(bsel[:], cd["bsel"][:])
        onesb = const.tile([P, 2], BF16)
        nc.sync.dma_start(onesb[:], cd["onesb"][:])
        cb16 = const.tile([2, P], BF16)
        nc.sync.dma_start(cb16[:], cd["cb16"][:])

        def vec128(name, src):
            t = const.tile([P, CCN], F32, name=name)
            nc.sync.dma_start(t[:], src.rearrange("(cc p) -> p cc", p=P))
            return t

        gam_sb = vec128("gam_sb", vd["gam"])
        bet_sb = vec128("bet_sb", vd["bet"])
        b2_sb = None
        if with_b2:
            b2_sb = vec128("b2_sb", vd["b2"])
        ut8 = const.tile([P, CCN], FP8)
        if with_t:
            nc.sync.dma_start(ut8[:], vd["u8"].rearrange("(cc p) -> p cc", p=P))

        M8 = persist.tile([P, CCN, C], FP8)      # M[c, c'] * SM
        W2T8 = persist.tile([P, CCN, C], FP8)    # W2^T[c, o] * SW2
        for cc in range(CCN):
            nc.sync.dma_start(M8[:, cc, :], m8v[:, cc, :])
            nc.sync.dma_start(W2T8[:, cc, :], w2v[:, cc, :])

        hn = persist.tile([P, CCN, N], FP8)      # GN(x), fp8
        q28 = persist.tile([P, CCN, NQ], FP8)    # q2 * SQ
        P2T8 = persist.tile([P, JCN, C], FP8)    # (W2 hn)^T * SPP, [j, o]
        t_part = const.tile([P, JCN], F32)       # t[j] laid out [p, jc]
        A_sb = const.tile([P, CCN], F32)
        B_sb = const.tile([P, CCN], F32)
        bnbuf = const.tile([P, CCN, NB, 6], F32)
        mv = const.tile([P, CCN, 2], F32)

        # ---- x load (resident) + GN stats chasing the DMA pieces ----
        xpool = tc.alloc_tile_pool(name="xres", bufs=1)
        xfull = xpool.tile([P, CCN, N], F32, name="xfull", tag="x")
        for nb in range(NB):
            for cc in range(CCN):
                nc.sync.dma_start(
                    xfull[:, cc, nb * 512:(nb + 1) * 512],
                    xv[:, cc, nb * 512:(nb + 1) * 512])
        for nb in range(NB):
            for cc in range(CCN):
                nc.vector.bn_stats(bnbuf[:, cc, nb, :],
                                   xfull[:, cc, nb * 512:(nb + 1) * 512])

        pearly = tc.alloc_tile_pool(name="pearly", bufs=3, space="PSUM")

        # ---- GN stat aggregation -> per-channel A, B ----
        for cc in range(CCN):
            nc.vector.bn_aggr(mv[:, cc, :],
                              bnbuf[:, cc, :, :].rearrange("p a b -> p (a b)"))
        stats8 = const.tile([P, 8], F32)
        nc.vector.tensor_copy(stats8[:, 0:4], mv[:, :, 0])
        nc.vector.tensor_mul(stats8[:, 4:8], mv[:, :, 0], mv[:, :, 0])
        nc.vector.tensor_add(stats8[:, 4:8], stats8[:, 4:8], mv[:, :, 1])
        gs_ps = pearly.tile([8, 8], F32, tag="big")
        nc.tensor.matmul(gs_ps[:], sel[:], stats8[:], start=True, stop=True)
        gs_sb = const.tile([8, 8], F32)
        nc.vector.tensor_copy(gs_sb[:], gs_ps[:])
        gvar = const.tile([8, 4], F32)
        nc.vector.tensor_mul(gvar[:], gs_sb[:, 0:4], gs_sb[:, 0:4])
        nc.vector.tensor_sub(gvar[:], gs_sb[:, 4:8], gvar[:])
        nc.vector.tensor_scalar_add(gvar[:], gvar[:], EPS)
        gsq = const.tile([8, 4], F32)
        nc.scalar.sqrt(gsq[:], gvar[:])
        grs2 = const.tile([8, 8], F32)
        nc.vector.tensor_copy(grs2[:, 0:4], gs_sb[:, 0:4])
        nc.vector.reciprocal(grs2[:, 4:8], gsq[:])
        bc_ps = pearly.tile([P, 8], F32, tag="big")
        nc.tensor.matmul(bc_ps[:], bsel[:], grs2[:], start=True, stop=True)
        nc.vector.tensor_mul(A_sb[:], gam_sb[:], bc_ps[:, 4:8])
        nc.vector.scalar_tensor_tensor(B_sb[:], bc_ps[:, 0:4], -1.0, A_sb[:],
                                       op0=ALU.mult, op1=ALU.mult)
        nc.vector.tensor_add(B_sb[:], B_sb[:], bet_sb[:])

        # ---- apply GN -> hn (fp8), reading the resident x ----
        for nb in range(NB):
            for cc in range(CCN):
                dst = hn[:, cc, nb * 512:(nb + 1) * 512]
                xsl = xfull[:, cc, nb * 512:(nb + 1) * 512]
                if cc < 2:
                    nc.vector.tensor_scalar(dst, xsl,
                                            A_sb[:, cc:cc + 1],
                                            B_sb[:, cc:cc + 1],
                                            op0=ALU.mult, op1=ALU.add)
                elif cc == 2:
                    nc.scalar.activation(dst, xsl, AF.Identity,
                                         bias=B_sb[:, cc:cc + 1],
                                         scale=A_sb[:, cc:cc + 1])
                else:
                    nc.gpsimd.tensor_scalar(dst, xsl,
                                            A_sb[:, cc:cc + 1],
                                            B_sb[:, cc:cc + 1],
                                            op0=ALU.mult, op1=ALU.add)

        # ---- q2[c', i] = sum_c M[c, c'] hn[c, i]  (i in 0:1024), DR fp8 ----
        for cch in range(CCN):
            for ih in range(2):
                q_ps = pearly.tile([P, 512], F32, name="q_ps", tag="big")
                for h in range(2):
                    nc.tensor.matmul(q_ps[:],
                                     M8[:, 2 * h:2 * h + 2,
                                        cch * P:(cch + 1) * P],
                                     hn[:, 2 * h:2 * h + 2,
                                        ih * 512:(ih + 1) * 512],
                                     start=(h == 0), stop=(h == 1),
                                     perf_mode=DR)
                nc.scalar.mul(q28[:, cch, ih * 512:(ih + 1) * 512], q_ps[:],
                              SQ / SM)

        # ---- P2T[j, o] = sum_c hn[c, j] W2T[c, o], DR fp8 ----
        for jc in range(JCN):
            p_ps = pearly.tile([P, 512], F32, name="p_ps", tag="big")
            for h in range(2):
                nc.tensor.matmul(p_ps[:],
                                 hn[:, 2 * h:2 * h + 2, jc * P:(jc + 1) * P],
                                 W2T8[:, 2 * h:2 * h + 2, :],
                                 start=(h == 0), stop=(h == 1),
                                 perf_mode=DR, skip_group_check=True)
            if jc % 2 == 0:
                nc.vector.tensor_scalar_mul(P2T8[:, jc, :], p_ps[:], SPP / SW2)
            else:
                nc.scalar.mul(P2T8[:, jc, :], p_ps[:], SPP / SW2)

        if with_t:
            # t[n] = sum_c' u[c'] hn[c', n] -> DRAM bounce -> t_part[p, jc]
            t_dram = dram.tile([N], F32)
            for nb in range(NB):
                t_ps = pearly.tile([1, 512], F32, name="t_ps", tag="big")
                for h in range(2):
                    nc.tensor.matmul(t_ps[:], ut8[:, 2 * h:2 * h + 2],
                                     hn[:, 2 * h:2 * h + 2,
                                        nb * 512:(nb + 1) * 512],
                                     start=(h == 0), stop=(h == 1),
                                     perf_mode=DR, skip_group_check=True)
                t_ch = evac.tile([1, 512], F32, name="t_ch", tag="tch", bufs=1)
                nc.scalar.mul(t_ch[:], t_ps[:], 1.0 / SU)
                nc.sync.dma_start(t_dram[nb * 512:(nb + 1) * 512], t_ch[:])
            nc.sync.dma_start(t_part[:], t_dram.rearrange("(jc p) -> p jc", p=P))

        pearly.release()

        # ---- attention ----
        # scores(ib0), scores(ib1) with exp -> fp8 aT pair tiles and the
        # softmax denominator accumulated on DVE; then per ib the fused
        # (attention x out-proj) GEMM accumulates P2T^T a over all 32 j
        # chunks (16 DoubleRow pairs), normalized by 1/(SPP*den) at evac.
        aTpool = tc.alloc_tile_pool(name="aT", bufs=34)
        patt = tc.alloc_tile_pool(name="patt", bufs=1, space="PSUM")
        aTs = {}
        dacc = [const.tile([P, 2, 512], F32, name=f"dacc{ib}")
                for ib in range(IBN)]
        for ib in range(IBN):
            i0 = ib * 512
            for jc in range(JCN):
                u, par = divmod(jc, 2)
                s_ps = patt.tile([P, 512], F32, name="s_ps", tag="s", bufs=3)
                for h in range(2):
                    nc.tensor.matmul(s_ps[:],
                                     hn[:, 2 * h:2 * h + 2, jc * P:(jc + 1) * P],
                                     q28[:, 2 * h:2 * h + 2, i0:i0 + 512],
                                     start=(h == 0), stop=(h == 1),
                                     perf_mode=DR)
                if par == 0:
                    aT_t = aTpool.tile([P, 2, 512], FP8, name="aT_t", tag="aT",
                                       bufs=34)
                    aTs[ib, u] = aT_t
                aT_t = aTs[ib, u]
                if with_t:
                    nc.scalar.activation(aT_t[:, par, :], s_ps[:], AF.Exp,
                                         bias=t_part[:, jc:jc + 1],
                                         scale=1.0 / SQ)
                else:
                    nc.scalar.activation(aT_t[:, par, :], s_ps[:], AF.Exp,
                                         scale=1.0 / SQ)
                if par == 1:
                    if u == 0:
                        nc.vector.tensor_copy(dacc[ib][:], aT_t[:])
                    else:
                        nc.vector.tensor_add(dacc[ib][:], dacc[ib][:], aT_t[:])

        bcsts = []
        for ib in range(IBN):
            i0 = ib * 512
            # denominator: fold pair lanes, partition-sum via PE, reciprocal,
            # broadcast scaled by 1/SPP
            daccb = evac.tile([P, 512], BF16, name="daccb", tag="daccb", bufs=1)
            nc.vector.tensor_add(daccb[:], dacc[ib][:, 0, :], dacc[ib][:, 1, :])
            den_ps = patt.tile([1, 512], F32, name="den_ps", tag="s", bufs=3)
            nc.tensor.matmul(den_ps[:], onesb[:, 0:1], daccb[:],
                             start=True, stop=True)
            recip = evac.tile([1, 512], BF16, name="recip", tag="recip")
            with nc.allow_low_precision(reason="bf16 1/denominator is ample"):
                nc.vector.reciprocal(recip[:], den_ps[:])
            bcst_ps = patt.tile([P, 512], F32, name="bcst_ps", tag="s", bufs=3)
            nc.tensor.matmul(bcst_ps[:], cb16[0:1, 0:P], recip[:],
                             start=True, stop=True)
            bcst_sb = const.tile([P, 512], F32, name=f"bcst{ib}")
            nc.scalar.copy(bcst_sb[:], bcst_ps[:])
            bcsts.append(bcst_sb)

            # fused (attention x out-proj): op[o, i] = sum_j P2T[j, o] a[j, i]
            op_ps = [patt.tile([P, 512], F32, name=f"op_ps{oc}", tag="av",
                               bufs=4) for oc in range(CCN)]
            for u in range(UCN):
                for oc in range(CCN):
                    nc.tensor.matmul(op_ps[oc][:],
                                     P2T8[:, 2 * u:2 * u + 2,
                                          oc * P:(oc + 1) * P],
                                     aTs[ib, u][:],
                                     start=(u == 0), stop=(u == UCN - 1),
                                     perf_mode=DR, skip_group_check=True)
            for oc in range(CCN):
                osb = evac.tile([P, 512], F32, name="osb", tag="osb")
                nc.vector.tensor_mul(osb[:], op_ps[oc][:], bcst_sb[:])
                xr = xfull[:, oc, i0:i0 + 512]
                if with_b2:
                    nc.vector.scalar_tensor_tensor(osb[:], osb[:],
                                                   b2_sb[:, oc:oc + 1], xr,
                                                   op0=ALU.add, op1=ALU.add)
                else:
                    nc.gpsimd.tensor_add(osb[:], osb[:], xr)
                nc.sync.dma_start(ov[:, oc, i0:i0 + 512], osb[:])

        aTpool.release()
        patt.release()
        xpool.release()


# ---------------- host side ----------------

_CACHED = {}


def _get_nc(with_t, with_b2):
    key = (with_t, with_b2)
    if key not in _CACHED:
        _CACHED[key] = build_program(with_t=with_t, with_b2=with_b2)
    return _CACHED[key]


def _host_constants():
    p = np.arange(P)
    sel = np.zeros((P, 8), np.float32)
    sel[p, p // GROUP] = 1.0 / GROUP
    bsel = np.zeros((8, P), np.float32)
    bsel[p // GROUP, p] = 1.0
    onesb = np.ones((P, 2), dtype=BF16NP)
    cb16 = np.full((2, P), 1.0 / SPP, dtype=BF16NP)
    return dict(sel=sel, bsel=bsel, onesb=onesb, cb16=cb16)


def _host_weights(wq, bq, wk, wv, bv, wo, bo):
    """Weights-only folds (input-independent): M, W2, b2, u."""
    wq = np.asarray(wq, np.float32)
    wk = np.asarray(wk, np.float32)
    wv = np.asarray(wv, np.float32)
    wo = np.asarray(wo, np.float32)
    M = (wq.T @ wk) * SCALE
    W2 = wo @ wv
    b2 = wo @ np.asarray(bv, np.float32) + np.asarray(bo, np.float32)
    u = (wk.T @ np.asarray(bq, np.float32)) * SCALE
    return (np.ascontiguousarray((M * SM).astype(FP8NP)),
            np.ascontiguousarray((W2.T * SW2).astype(FP8NP)),
            b2.astype(np.float32),
            (u * SU).astype(FP8NP))


def kernel(x, gn_scale, gn_bias, wq, bq, wk, bk, wv, bv, wo, bo):
    from concourse.bass_utils import run_bass_kernel_spmd

    m8, w2t8, b2, u8 = _host_weights(wq, bq, wk, wv, bv, wo, bo)
    with_t = bool(np.any(np.asarray(bq, np.float32) != 0))
    with_b2 = bool(np.any(b2 != 0))
    nc = _get_nc(with_t, with_b2)
    consts = _host_constants()
    xr = np.ascontiguousarray(np.asarray(x, np.float32).reshape(2, C, N))
    shared = dict(
        m8=m8, w2t8=w2t8, b2=b2, u8=u8,
        gamma=np.asarray(gn_scale, np.float32),
        beta=np.asarray(gn_bias, np.float32),
        **consts,
    )
    in_maps = []
    for core in range(8):
        b, qc = divmod(core, 4)
        perm_x = np.concatenate(
            [xr[b][:, qc * NQ:(qc + 1) * NQ],
             np.delete(xr[b], np.s_[qc * NQ:(qc + 1) * NQ], axis=1)], axis=1)
        in_maps.append({"xp": np.ascontiguousarray(perm_x), **shared})

    res = run_bass_kernel_spmd(nc, in_maps, core_ids=list(range(8)))
    y = np.empty((2, C, N), np.float32)
    for core in range(8):
        b, qc = divmod(core, 4)
        y[b][:, qc * NQ:(qc + 1) * NQ] = res.results[core]["out"]
    return y.reshape(2, C, 64, 64)


# revision 16
# speedup vs baseline: 1.6146x; 1.0257x over previous
"""Trainium2 Bass kernel for an AttnBlock (GroupNorm -> single-head attention
-> out-proj -> residual) on x[2, 512, 64, 64].

Sharding: 8 cores = batch(2) x query-chunk(4).  Each core receives its batch's
full x with its own 1024 query columns permuted to the front (GroupNorm stats
and softmax sums over spatial positions are permutation invariant), computes
GN for all 4096 positions, and attention for its 1024 queries.

Weight algebra is folded HOST-side (weights-only transforms, O(C^2)):
  M   = wq^T wk * c^-0.5      so scores[j,i] = hn_j^T M hn_i (+ t[j] terms)
  W2  = wo wv                 so out = W2 (hn A) / den + b2 + x
  b2  = wo bv + bo
The device computes, all in fp8(e4m3) DoubleRow matmuls with fp32 PSUM:
  q2   = M^T hn               (own 1024 queries)
  P2T  = (W2 hn)^T            [j, o] orientation, one GEMM, no transposes
  s    = hn^T q2 ; a = exp(s) ; den = sum_j a
  out  = (P2T^T a) / den + x  (attention + out-proj fused in ONE GEMM)
"""

import numpy as np
import ml_dtypes

import concourse.bass as bass
import concourse.tile as tile
from concourse import mybir

P = 128
C = 512
N = 4096
NQ = 1024          # queries per core
CCN = 4            # channel chunks of 128
NB = 8             # n chunks of 512
JCN = 32           # j chunks of 128
UCN = 16           # j chunk pairs (DoubleRow)
IBN = 2            # i blocks of 512 per core
SCALE = float(C) ** -0.5
EPS = 1e-6
GROUP = 16         # channels per group

# fp8 scale plan (see module docstring algebra):
SM = 1024.0        # M8 = fp8(M * SM)
SQ = 64.0          # q28 = fp8(q2 * SQ) = fp8(q_psum * SQ/SM)
SW2 = 512.0        # W2T8 = fp8(W2^T * SW2)
SPP = 16.0         # P2T8 = fp8(P2T * SPP) = fp8(p_psum * SPP/SW2)
SU = 64.0          # u8 = fp8(u * SU) for the t-vector path

F32 = mybir.dt.float32
BF16 = mybir.dt.bfloat16
FP8 = mybir.dt.float8e4
AF = mybir.ActivationFunctionType
ALU = mybir.AluOpType
DR = mybir.MatmulPerfMode.DoubleRow
BF16NP = ml_dtypes.bfloat16
FP8NP = ml_dtypes.float8_e4m3

_WAIT_LIMIT = 1


def _split_excess_waits(nc):
    """This walrus build rejects multi-wait sync on one instruction.  Move
    excess waits onto same-engine NoOps inserted just before the offending
    instruction; engine queues (and the SP DMA-trigger stream) are FIFO, so
    semantics are preserved."""
    counter = 0
    for f in nc.m.functions:
        for bb in f.blocks:
            insts = bb.instructions
            out = []
            for ins in insts:
                si = ins.sync_info
                waits = list(si.on_wait) if si and si.on_wait else []
                if len(waits) > _WAIT_LIMIT:
                    si.on_wait = waits[-_WAIT_LIMIT:]
                    extra = waits[:-_WAIT_LIMIT]
                    for i in range(0, len(extra), _WAIT_LIMIT):
                        nop = mybir.InstNoOp(
                            name=f"I-wsplit-{counter}", ins=[], outs=[])
                        counter += 1
                        nop.engine = ins.engine
                        nop.sync_info = mybir.SyncInfo(
                            on_wait=extra[i:i + _WAIT_LIMIT], on_update=[])
                        out.append(nop)
                out.append(ins)
            insts[:] = out


def build_program(with_t=False, with_b2=False, split_waits=True):
    nc = bass.Bass("TRN2", target_bir_lowering=False, debug=False)

    xp = nc.dram_tensor("xp", [C, N], F32, kind="ExternalInput").ap()
    m8_d = nc.dram_tensor("m8", [C, C], FP8, kind="ExternalInput").ap()
    w2t8_d = nc.dram_tensor("w2t8", [C, C], FP8, kind="ExternalInput").ap()
    u8_d = nc.dram_tensor("u8", [C], FP8, kind="ExternalInput").ap()
    b2_d = nc.dram_tensor("b2", [C], F32, kind="ExternalInput").ap()
    gam_d = nc.dram_tensor("gamma", [C], F32, kind="ExternalInput").ap()
    bet_d = nc.dram_tensor("beta", [C], F32, kind="ExternalInput").ap()
    sel_d = nc.dram_tensor("sel", [P, 8], F32, kind="ExternalInput").ap()
    bsel_d = nc.dram_tensor("bsel", [8, P], F32, kind="ExternalInput").ap()
    ones8_d = nc.dram_tensor("ones8", [P, 2 * P], FP8, kind="ExternalInput").ap()
    out_d = nc.dram_tensor("out", [C, NQ], F32, kind="ExternalOutput").ap()

    xv = xp.rearrange("(cc p) n -> p cc n", p=P)
    m8v = m8_d.rearrange("(cc p) o -> p cc o", p=P)
    w2v = w2t8_d.rearrange("(cc p) o -> p cc o", p=P)
    ov = out_d.rearrange("(oc p) n -> p oc n", p=P)

    with tile.TileContext(nc) as tc:
        _emit(nc, tc, xv, ov, m8v, w2v,
              dict(u8=u8_d, b2=b2_d, gam=gam_d, bet=bet_d),
              dict(sel=sel_d, bsel=bsel_d, ones8=ones8_d),
              with_t=with_t, with_b2=with_b2)
    if split_waits:
        _split_excess_waits(nc)
    return nc


def _emit(nc, tc, xv, ov, m8v, w2v, vd, cd, with_t, with_b2):
    from contextlib import ExitStack
    ctx = ExitStack()
    with ctx:
        const = ctx.enter_context(tc.tile_pool(name="const", bufs=1))
        persist = ctx.enter_context(tc.tile_pool(name="persist", bufs=1))
        evac = ctx.enter_context(tc.tile_pool(name="evac", bufs=2))
        dram = ctx.enter_context(tc.tile_pool(name="dram", bufs=1, space="DRAM"))

        # ---- constants / small vectors ----
        sel = const.tile([P, 8], F32)
        nc.sync.dma_start(sel[:], cd["sel"][:])
        bsel = const.tile([8, P], F32)
        nc.sync.dma_start(bsel[:], cd["bsel"][:])
        ones8 = const.tile([P, 2, P], FP8)
        nc.sync.dma_start(ones8[:], cd["ones8"].rearrange("p (a b) -> p a b", a=2))

        def vec128(name, src):
            t = const.tile([P, CCN], F32, name=name)
            nc.sync.dma_start(t[:], src.rearrange("(cc p) -> p cc", p=P))
            return t

        gam_sb = vec128("gam_sb", vd["gam"])
        bet_sb = vec128("bet_sb", vd["bet"])
        b2_sb = None
        if with_b2:
            b2_sb = vec128("b2_sb", vd["b2"])
        ut8 = const.tile([P, CCN], FP8)
        if with_t:
            nc.sync.dma_start(ut8[:], vd["u8"].rearrange("(cc p) -> p cc", p=P))

        M8 = persist.tile([P, CCN, C], FP8)      # M[c, c'] * SM
        W2T8 = persist.tile([P, CCN, C], FP8)    # W2^T[c, o] * SW2
        for cc in range(CCN):
            nc.sync.dma_start(M8[:, cc, :], m8v[:, cc, :])
            nc.sync.dma_start(W2T8[:, cc, :], w2v[:, cc, :])

        hn = persist.tile([P, CCN, N], FP8)      # GN(x), fp8
        q28 = persist.tile([P, CCN, NQ], FP8)    # q2 * SQ
        P2T8 = persist.tile([P, JCN, C], FP8)    # (W2 hn)^T * SPP, [j, o]
        t_part = const.tile([P, JCN], F32)       # t[j] laid out [p, jc]
        A_sb = const.tile([P, CCN], F32)
        B_sb = const.tile([P, CCN], F32)
        bnbuf = const.tile([P, CCN, NB, 6], F32)
        mv = const.tile([P, CCN, 2], F32)

        # ---- x load (resident) + GN stats chasing the DMA pieces ----
        # Ramped piece sizes: the 16 HWDGE queues run in parallel, so the
        # first wave completes at (piece bytes)/(per-queue BW).  Small leading
        # pieces let bn_stats start within a few us instead of ~16us.
        xpool = tc.alloc_tile_pool(name="xres", bufs=1)
        xfull = xpool.tile([P, CCN, N], F32, name="xfull", tag="x")
        bounds = [0, 128, 256, 512, 1024, 1536, 2048, 2560, 3072, 3584, 4096]
        for h in range(len(bounds) - 1):
            for cc in range(CCN):
                nc.sync.dma_start(
                    xfull[:, cc, bounds[h]:bounds[h + 1]],
                    xv[:, cc, bounds[h]:bounds[h + 1]])
        for nb in range(NB):
            for cc in range(CCN):
                nc.vector.bn_stats(bnbuf[:, cc, nb, :],
                                   xfull[:, cc, nb * 512:(nb + 1) * 512])

        pearly = tc.alloc_tile_pool(name="pearly", bufs=3, space="PSUM")

        # ---- GN stat aggregation -> per-channel A, B ----
        for cc in range(CCN):
            nc.vector.bn_aggr(mv[:, cc, :],
                              bnbuf[:, cc, :, :].rearrange("p a b -> p (a b)"))
        stats8 = const.tile([P, 8], F32)
        nc.vector.tensor_copy(stats8[:, 0:4], mv[:, :, 0])
        nc.vector.tensor_mul(stats8[:, 4:8], mv[:, :, 0], mv[:, :, 0])
        nc.vector.tensor_add(stats8[:, 4:8], stats8[:, 4:8], mv[:, :, 1])
        gs_ps = pearly.tile([8, 8], F32, tag="big")
        nc.tensor.matmul(gs_ps[:], sel[:], stats8[:], start=True, stop=True)
        gs_sb = const.tile([8, 8], F32)
        nc.vector.tensor_copy(gs_sb[:], gs_ps[:])
        gvar = const.tile([8, 4], F32)
        nc.vector.tensor_mul(gvar[:], gs_sb[:, 0:4], gs_sb[:, 0:4])
        nc.vector.tensor_sub(gvar[:], gs_sb[:, 4:8], gvar[:])
        nc.vector.tensor_scalar_add(gvar[:], gvar[:], EPS)
        gsq = const.tile([8, 4], F32)
        nc.scalar.sqrt(gsq[:], gvar[:])
        grs2 = const.tile([8, 8], F32)
        nc.vector.tensor_copy(grs2[:, 0:4], gs_sb[:, 0:4])
        nc.vector.reciprocal(grs2[:, 4:8], gsq[:])
        bc_ps = pearly.tile([P, 8], F32, tag="big")
        nc.tensor.matmul(bc_ps[:], bsel[:], grs2[:], start=True, stop=True)
        nc.vector.tensor_mul(A_sb[:], gam_sb[:], bc_ps[:, 4:8])
        nc.vector.scalar_tensor_tensor(B_sb[:], bc_ps[:, 0:4], -1.0, A_sb[:],
                                       op0=ALU.mult, op1=ALU.mult)
        nc.vector.tensor_add(B_sb[:], B_sb[:], bet_sb[:])

        # ---- apply GN -> hn (fp8), reading the resident x ----
        for nb in range(NB):
            for cc in range(CCN):
                dst = hn[:, cc, nb * 512:(nb + 1) * 512]
                xsl = xfull[:, cc, nb * 512:(nb + 1) * 512]
                if cc == 0:
                    nc.vector.tensor_scalar(dst, xsl,
                                            A_sb[:, cc:cc + 1],
                                            B_sb[:, cc:cc + 1],
                                            op0=ALU.mult, op1=ALU.add)
                elif cc == 1:
                    nc.scalar.activation(dst, xsl, AF.Identity,
                                         bias=B_sb[:, cc:cc + 1],
                                         scale=A_sb[:, cc:cc + 1])
                else:
                    nc.gpsimd.tensor_scalar(dst, xsl,
                                            A_sb[:, cc:cc + 1],
                                            B_sb[:, cc:cc + 1],
                                            op0=ALU.mult, op1=ALU.add)

        # ---- q2[c', i] = sum_c M[c, c'] hn[c, i]  (i in 0:1024), DR fp8 ----
        for cch in range(CCN):
            for ih in range(2):
                q_ps = pearly.tile([P, 512], F32, name="q_ps", tag="big")
                for h in range(2):
                    nc.tensor.matmul(q_ps[:],
                                     M8[:, 2 * h:2 * h + 2,
                                        cch * P:(cch + 1) * P],
                                     hn[:, 2 * h:2 * h + 2,
                                        ih * 512:(ih + 1) * 512],
                                     start=(h == 0), stop=(h == 1),
                                     perf_mode=DR)
                nc.scalar.mul(q28[:, cch, ih * 512:(ih + 1) * 512], q_ps[:],
                              SQ / SM)

        # ---- P2T[j, o] = sum_c hn[c, j] W2T[c, o], DR fp8 ----
        for jc in range(JCN):
            p_ps = pearly.tile([P, 512], F32, name="p_ps", tag="big")
            for h in range(2):
                nc.tensor.matmul(p_ps[:],
                                 hn[:, 2 * h:2 * h + 2, jc * P:(jc + 1) * P],
                                 W2T8[:, 2 * h:2 * h + 2, :],
                                 start=(h == 0), stop=(h == 1),
                                 perf_mode=DR, skip_group_check=True)
            if jc % 2 == 0:
                nc.vector.tensor_scalar_mul(P2T8[:, jc, :], p_ps[:], SPP / SW2)
            else:
                nc.scalar.mul(P2T8[:, jc, :], p_ps[:], SPP / SW2)

        if with_t:
            # t[n] = sum_c' u[c'] hn[c', n] -> DRAM bounce -> t_part[p, jc]
            t_dram = dram.tile([N], F32)
            for nb in range(NB):
                t_ps = pearly.tile([1, 512], F32, name="t_ps", tag="big")
                for h in range(2):
                    nc.tensor.matmul(t_ps[:], ut8[:, 2 * h:2 * h + 2],
                                     hn[:, 2 * h:2 * h + 2,
                                        nb * 512:(nb + 1) * 512],
                                     start=(h == 0), stop=(h == 1),
                                     perf_mode=DR, skip_group_check=True)
                t_ch = evac.tile([1, 512], F32, name="t_ch", tag="tch", bufs=1)
                nc.scalar.mul(t_ch[:], t_ps[:], 1.0 / SU)
                nc.sync.dma_start(t_dram[nb * 512:(nb + 1) * 512], t_ch[:])
            nc.sync.dma_start(t_part[:], t_dram.rearrange("(jc p) -> p jc", p=P))

        pearly.release()

        # ---- attention ----
        # scores(ib0), scores(ib1) with exp -> fp8 aT pair tiles; then per
        # i-block: softmax denominator summed ON PE (ones8 DoubleRow against
        # each aT pair tile, accumulating a [1, 512] PSUM), the fused
        # (attention x out-proj) GEMM accumulating P2T^T a over all 32 j
        # chunks, and evacuation normalized by 1/(SPP*den).
        aTpool = tc.alloc_tile_pool(name="aT", bufs=34)
        patt = tc.alloc_tile_pool(name="patt", bufs=1, space="PSUM")
        aTs = {}
        for ib in range(IBN):
            i0 = ib * 512
            for jc in range(JCN):
                u, par = divmod(jc, 2)
                s_ps = patt.tile([P, 512], F32, name="s_ps", tag="s", bufs=3)
                for h in range(2):
                    nc.tensor.matmul(s_ps[:],
                                     hn[:, 2 * h:2 * h + 2, jc * P:(jc + 1) * P],
                                     q28[:, 2 * h:2 * h + 2, i0:i0 + 512],
                                     start=(h == 0), stop=(h == 1),
                                     perf_mode=DR)
                if par == 0:
                    aT_t = aTpool.tile([P, 2, 512], FP8, name="aT_t", tag="aT",
                                       bufs=34)
                    aTs[ib, u] = aT_t
                aT_t = aTs[ib, u]
                if with_t:
                    nc.scalar.activation(aT_t[:, par, :], s_ps[:], AF.Exp,
                                         bias=t_part[:, jc:jc + 1],
                                         scale=1.0 / SQ)
                else:
                    nc.scalar.activation(aT_t[:, par, :], s_ps[:], AF.Exp,
                                         scale=1.0 / SQ)

        for ib in range(IBN):
            # den[i] = sum_j a[j, i] on PE, broadcast to all 128 partitions
            # by the all-ones stationary; reciprocal on DVE overlaps the
            # AVproj matmuls that follow.
            den_ps = patt.tile([P, 512], F32, name=f"den_ps{ib}", tag="s",
                               bufs=3)
            for u in range(UCN):
                nc.tensor.matmul(den_ps[:], ones8[:], aTs[ib, u][:],
                                 start=(u == 0), stop=(u == UCN - 1),
                                 perf_mode=DR, skip_group_check=True)
            recip = const.tile([P, 512], BF16, name=f"recip{ib}")
            with nc.allow_low_precision(reason="bf16 1/denominator is ample"):
                nc.vector.reciprocal(recip[:], den_ps[:])

            # fused (attention x out-proj): op[o, i] = sum_j P2T[j, o] a[j, i]
            op_ps = [patt.tile([P, 512], F32, name=f"op_ps{oc}", tag="av",
                               bufs=4) for oc in range(CCN)]
            for u in range(UCN):
                for oc in range(CCN):
                    nc.tensor.matmul(op_ps[oc][:],
                                     P2T8[:, 2 * u:2 * u + 2,
                                          oc * P:(oc + 1) * P],
                                     aTs[ib, u][:],
                                     start=(u == 0), stop=(u == UCN - 1),
                                     perf_mode=DR, skip_group_check=True)

            i0 = ib * 512
            for oc in range(CCN):
                osb = evac.tile([P, 512], F32, name="osb", tag="osb")
                nc.vector.scalar_tensor_tensor(osb[:], op_ps[oc][:],
                                               1.0 / SPP, recip[:],
                                               op0=ALU.mult, op1=ALU.mult)
                xr = xfull[:, oc, i0:i0 + 512]
                if with_b2:
                    nc.vector.scalar_tensor_tensor(osb[:], osb[:],
                                                   b2_sb[:, oc:oc + 1], xr,
                                                   op0=ALU.add, op1=ALU.add)
                elif oc % 2 == 0:
                    nc.gpsimd.tensor_add(osb[:], osb[:], xr)
                else:
                    nc.vector.tensor_add(osb[:], osb[:], xr)
                nc.sync.dma_start(ov[:, oc, i0:i0 + 512], osb[:])

        aTpool.release()
        patt.release()
        xpool.release()


# ---------------- host side ----------------

_CACHED = {}


def _get_nc(with_t, with_b2):
    key = (with_t, with_b2)
    if key not in _CACHED:
        _CACHED[key] = build_program(with_t=with_t, with_b2=with_b2)
    return _CACHED[key]


def _host_constants():
    p = np.arange(P)
    sel = np.zeros((P, 8), np.float32)
    sel[p, p // GROUP] = 1.0 / GROUP
    bsel = np.zeros((8, P), np.float32)
    bsel[p // GROUP, p] = 1.0
    ones8 = np.ones((P, 2 * P), dtype=FP8NP)
    return dict(sel=sel, bsel=bsel, ones8=ones8)


def _host_weights(wq, bq, wk, wv, bv, wo, bo):
    """Weights-only folds (input-independent): M, W2, b2, u."""
    wq = np.asarray(wq, np.float32)
    wk = np.asarray(wk, np.float32)
    wv = np.asarray(wv, np.float32)
    wo = np.asarray(wo, np.float32)
    M = (wq.T @ wk) * SCALE
    W2 = wo @ wv
    b2 = wo @ np.asarray(bv, np.float32) + np.asarray(bo, np.float32)
    u = (wk.T @ np.asarray(bq, np.float32)) * SCALE
    return (np.ascontiguousarray((M * SM).astype(FP8NP)),
            np.ascontiguousarray((W2.T * SW2).astype(FP8NP)),
            b2.astype(np.float32),
            (u * SU).astype(FP8NP))


def kernel(x, gn_scale, gn_bias, wq, bq, wk, bk, wv, bv, wo, bo):
    from concourse.bass_utils import run_bass_kernel_spmd

    m8, w2t8, b2, u8 = _host_weights(wq, bq, wk, wv, bv, wo, bo)
    with_t = bool(np.any(np.asarray(bq, np.float32) != 0))
    with_b2 = bool(np.any(b2 != 0))
    nc = _get_nc(with_t, with_b2)
    consts = _host_constants()
    xr = np.ascontiguousarray(np.asarray(x, np.float32).reshape(2, C, N))
    shared = dict(
        m8=m8, w2t8=w2t8, b2=b2, u8=u8,
        gamma=np.asarray(gn_scale, np.float32),
        beta=np.asarray(gn_bias, np.float32),
        **consts,
    )
    in_maps = []
    for core in range(8):
        b, qc = divmod(core, 4)
        perm_x = np.concatenate(
            [xr[b][:, qc * NQ:(qc + 1) * NQ],
             np.delete(xr[b], np.s_[qc * NQ:(qc + 1) * NQ], axis=1)], axis=1)
        in_maps.append({"xp": np.ascontiguousarray(perm_x), **shared})

    res = run_bass_kernel_spmd(nc, in_maps, core_ids=list(range(8)))
    y = np.empty((2, C, N), np.float32)
    for core in range(8):
        b, qc = divmod(core, 4)
        y[b][:, qc * NQ:(qc + 1) * NQ] = res.results[core]["out"]
    return y.reshape(2, C, 64, 64)


# revision 19
# speedup vs baseline: 1.7356x; 1.0749x over previous
"""Trainium2 Bass kernel for an AttnBlock (GroupNorm -> single-head attention
-> out-proj -> residual) on x[2, 512, 64, 64].

Sharding: 8 cores = batch(2) x query-chunk(4).  Each core receives its batch's
full x with its own 1024 query columns permuted to the front (GroupNorm stats
and softmax sums over spatial positions are permutation invariant), computes
GN for all 4096 positions, and attention for its 1024 queries.

Weight algebra is folded HOST-side (weights-only transforms, O(C^2)):
  M   = wq^T wk * c^-0.5      so scores[j,i] = hn_j^T M hn_i (+ t[j] terms)
  W2  = wo wv                 so out = W2 (hn A) / den + b2 + x
  b2  = wo bv + bo
The device computes, all in fp8(e4m3) DoubleRow matmuls with fp32 PSUM:
  q2   = M^T hn               (own 1024 queries)
  P2T  = (W2 hn)^T            [j, o] orientation, one GEMM, no transposes
  s    = hn^T q2 ; a = exp(s) ; den = sum_j a
  out  = (P2T^T a) / den + x  (attention + out-proj fused in ONE GEMM)
"""

import numpy as np
import ml_dtypes

import concourse.bass as bass
import concourse.tile as tile
from concourse import mybir

P = 128
C = 512
N = 4096
NQ = 1024          # queries per core
CCN = 4            # channel chunks of 128
NB = 8             # n chunks of 512
JCN = 32           # j chunks of 128
UCN = 16           # j chunk pairs (DoubleRow)
IBN = 2            # i blocks of 512 per core
SCALE = float(C) ** -0.5
EPS = 1e-6
GROUP = 16         # channels per group

# fp8 scale plan (see module docstring algebra):
SM = 1024.0        # M8 = fp8(M * SM)
SQ = 64.0          # q28 = fp8(q2 * SQ) = fp8(q_psum * SQ/SM)
SW2 = 512.0        # W2T8 = fp8(W2^T * SW2)
SPP = 16.0         # P2T8 = fp8(P2T * SPP) = fp8(p_psum * SPP/SW2)
SU = 64.0          # u8 = fp8(u * SU) for the t-vector path

F32 = mybir.dt.float32
BF16 = mybir.dt.bfloat16
FP8 = mybir.dt.float8e4
AF = mybir.ActivationFunctionType
ALU = mybir.AluOpType
DR = mybir.MatmulPerfMode.DoubleRow
BF16NP = ml_dtypes.bfloat16
FP8NP = ml_dtypes.float8_e4m3

_WAIT_LIMIT = 1


def _split_excess_waits(nc):
    """This walrus build rejects multi-wait sync on one instruction.  Move
    excess waits onto same-engine NoOps inserted just before the offending
    instruction; engine queues (and the SP DMA-trigger stream) are FIFO, so
    semantics are preserved."""
    counter = 0
    for f in nc.m.functions:
        for bb in f.blocks:
            insts = bb.instructions
            out = []
            for ins in insts:
                si = ins.sync_info
                waits = list(si.on_wait) if si and si.on_wait else []
                if len(waits) > _WAIT_LIMIT:
                    si.on_wait = waits[-_WAIT_LIMIT:]
                    extra = waits[:-_WAIT_LIMIT]
                    for i in range(0, len(extra), _WAIT_LIMIT):
                        nop = mybir.InstNoOp(
                            name=f"I-wsplit-{counter}", ins=[], outs=[])
                        counter += 1
                        nop.engine = ins.engine
                        nop.sync_info = mybir.SyncInfo(
                            on_wait=extra[i:i + _WAIT_LIMIT], on_update=[])
                        out.append(nop)
                out.append(ins)
            insts[:] = out


def build_program(with_t=False, with_b2=False, split_waits=True):
    nc = bass.Bass("TRN2", target_bir_lowering=False, debug=False)

    xp = nc.dram_tensor("xp", [C, N], BF16, kind="ExternalInput").ap()
    m8_d = nc.dram_tensor("m8", [C, C], FP8, kind="ExternalInput").ap()
    w2t8_d = nc.dram_tensor("w2t8", [C, C], FP8, kind="ExternalInput").ap()
    u8_d = nc.dram_tensor("u8", [C], FP8, kind="ExternalInput").ap()
    b2_d = nc.dram_tensor("b2", [C], F32, kind="ExternalInput").ap()
    gam_d = nc.dram_tensor("gamma", [C], F32, kind="ExternalInput").ap()
    bet_d = nc.dram_tensor("beta", [C], F32, kind="ExternalInput").ap()
    sel_d = nc.dram_tensor("sel", [P, 8], F32, kind="ExternalInput").ap()
    bsel_d = nc.dram_tensor("bsel", [8, P], F32, kind="ExternalInput").ap()
    ones8_d = nc.dram_tensor("ones8", [P, 2 * P], FP8, kind="ExternalInput").ap()
    out_d = nc.dram_tensor("out", [C, NQ], BF16, kind="ExternalOutput").ap()

    xv = xp.rearrange("(cc p) n -> p cc n", p=P)
    m8v = m8_d.rearrange("(cc p) o -> p cc o", p=P)
    w2v = w2t8_d.rearrange("(cc p) o -> p cc o", p=P)
    ov = out_d.rearrange("(oc p) n -> p oc n", p=P)

    with tile.TileContext(nc) as tc:
        _emit(nc, tc, xv, ov, m8v, w2v,
              dict(u8=u8_d, b2=b2_d, gam=gam_d, bet=bet_d),
              dict(sel=sel_d, bsel=bsel_d, ones8=ones8_d),
              with_t=with_t, with_b2=with_b2)
    if split_waits:
        _split_excess_waits(nc)
    return nc


def _emit(nc, tc, xv, ov, m8v, w2v, vd, cd, with_t, with_b2):
    from contextlib import ExitStack
    ctx = ExitStack()
    with ctx:
        const = ctx.enter_context(tc.tile_pool(name="const", bufs=1))
        persist = ctx.enter_context(tc.tile_pool(name="persist", bufs=1))
        evac = ctx.enter_context(tc.tile_pool(name="evac", bufs=2))
        dram = ctx.enter_context(tc.tile_pool(name="dram", bufs=1, space="DRAM"))

        # ---- constants / small vectors ----
        sel = const.tile([P, 8], F32)
        nc.sync.dma_start(sel[:], cd["sel"][:])
        bsel = const.tile([8, P], F32)
        nc.sync.dma_start(bsel[:], cd["bsel"][:])
        ones8 = const.tile([P, 2, P], FP8)
        nc.sync.dma_start(ones8[:], cd["ones8"].rearrange("p (a b) -> p a b", a=2))

        def vec128(name, src):
            t = const.tile([P, CCN], F32, name=name)
            nc.sync.dma_start(t[:], src.rearrange("(cc p) -> p cc", p=P))
            return t

        gam_sb = vec128("gam_sb", vd["gam"])
        bet_sb = vec128("bet_sb", vd["bet"])
        b2_sb = None
        if with_b2:
            b2_sb = vec128("b2_sb", vd["b2"])
        ut8 = const.tile([P, CCN], FP8)
        if with_t:
            nc.sync.dma_start(ut8[:], vd["u8"].rearrange("(cc p) -> p cc", p=P))

        M8 = persist.tile([P, CCN, C], FP8)      # M[c, c'] * SM
        W2T8 = persist.tile([P, CCN, C], FP8)    # W2^T[c, o] * SW2
        for cc in range(CCN):
            nc.sync.dma_start(M8[:, cc, :], m8v[:, cc, :])
            nc.sync.dma_start(W2T8[:, cc, :], w2v[:, cc, :])

        hn = persist.tile([P, CCN, N], FP8)      # GN(x), fp8
        q28 = persist.tile([P, CCN, NQ], FP8)    # q2 * SQ
        P2T8 = persist.tile([P, JCN, C], FP8)    # (W2 hn)^T * SPP, [j, o]
        t_part = const.tile([P, JCN], F32)       # t[j] laid out [p, jc]
        A_sb = const.tile([P, CCN], F32)
        B_sb = const.tile([P, CCN], F32)
        bnbuf = const.tile([P, CCN, NB, 6], F32)
        mv = const.tile([P, CCN, 2], F32)

        # ---- x load (resident) + GN stats chasing the DMA pieces ----
        # Ramped piece sizes: the 16 HWDGE queues run in parallel, so the
        # first wave completes at (piece bytes)/(per-queue BW).  Small leading
        # pieces let bn_stats start within a few us instead of ~16us.
        xpool = tc.alloc_tile_pool(name="xres", bufs=1)
        xfull = xpool.tile([P, CCN, N], BF16, name="xfull", tag="x")
        for nb in range(NB):
            for cc in range(CCN):
                nc.sync.dma_start(
                    xfull[:, cc, nb * 512:(nb + 1) * 512],
                    xv[:, cc, nb * 512:(nb + 1) * 512])
        for nb in range(NB):
            for cc in range(CCN):
                nc.vector.bn_stats(bnbuf[:, cc, nb, :],
                                   xfull[:, cc, nb * 512:(nb + 1) * 512])

        pearly = tc.alloc_tile_pool(name="pearly", bufs=3, space="PSUM")

        # ---- GN stat aggregation -> per-channel A, B ----
        for cc in range(CCN):
            nc.vector.bn_aggr(mv[:, cc, :],
                              bnbuf[:, cc, :, :].rearrange("p a b -> p (a b)"))
        stats8 = const.tile([P, 8], F32)
        nc.vector.tensor_copy(stats8[:, 0:4], mv[:, :, 0])
        nc.vector.tensor_mul(stats8[:, 4:8], mv[:, :, 0], mv[:, :, 0])
        nc.vector.tensor_add(stats8[:, 4:8], stats8[:, 4:8], mv[:, :, 1])
        gs_ps = pearly.tile([8, 8], F32, tag="big")
        nc.tensor.matmul(gs_ps[:], sel[:], stats8[:], start=True, stop=True)
        gs_sb = const.tile([8, 8], F32)
        nc.vector.tensor_copy(gs_sb[:], gs_ps[:])
        gvar = const.tile([8, 4], F32)
        nc.vector.tensor_mul(gvar[:], gs_sb[:, 0:4], gs_sb[:, 0:4])
        nc.vector.tensor_sub(gvar[:], gs_sb[:, 4:8], gvar[:])
        nc.vector.tensor_scalar_add(gvar[:], gvar[:], EPS)
        gsq = const.tile([8, 4], F32)
        nc.scalar.sqrt(gsq[:], gvar[:])
        grs2 = const.tile([8, 8], F32)
        nc.vector.tensor_copy(grs2[:, 0:4], gs_sb[:, 0:4])
        nc.vector.reciprocal(grs2[:, 4:8], gsq[:])
        bc_ps = pearly.tile([P, 8], F32, tag="big")
        nc.tensor.matmul(bc_ps[:], bsel[:], grs2[:], start=True, stop=True)
        nc.vector.tensor_mul(A_sb[:], gam_sb[:], bc_ps[:, 4:8])
        nc.vector.scalar_tensor_tensor(B_sb[:], bc_ps[:, 0:4], -1.0, A_sb[:],
                                       op0=ALU.mult, op1=ALU.mult)
        nc.vector.tensor_add(B_sb[:], B_sb[:], bet_sb[:])

        # ---- apply GN -> hn (fp8), reading the resident x ----
        for nb in range(NB):
            for cc in range(CCN):
                dst = hn[:, cc, nb * 512:(nb + 1) * 512]
                xsl = xfull[:, cc, nb * 512:(nb + 1) * 512]
                if cc == 0:
                    nc.vector.tensor_scalar(dst, xsl,
                                            A_sb[:, cc:cc + 1],
                                            B_sb[:, cc:cc + 1],
                                            op0=ALU.mult, op1=ALU.add)
                elif cc == 1:
                    nc.scalar.activation(dst, xsl, AF.Identity,
                                         bias=B_sb[:, cc:cc + 1],
                                         scale=A_sb[:, cc:cc + 1])
                else:
                    nc.gpsimd.tensor_scalar(dst, xsl,
                                            A_sb[:, cc:cc + 1],
                                            B_sb[:, cc:cc + 1],
                                            op0=ALU.mult, op1=ALU.add)

        # ---- q2[c', i] = sum_c M[c, c'] hn[c, i]  (i in 0:1024), DR fp8 ----
        for cch in range(CCN):
            for ih in range(2):
                q_ps = pearly.tile([P, 512], F32, name="q_ps", tag="big")
                for h in range(2):
                    nc.tensor.matmul(q_ps[:],
                                     M8[:, 2 * h:2 * h + 2,
                                        cch * P:(cch + 1) * P],
                                     hn[:, 2 * h:2 * h + 2,
                                        ih * 512:(ih + 1) * 512],
                                     start=(h == 0), stop=(h == 1),
                                     perf_mode=DR)
                nc.scalar.mul(q28[:, cch, ih * 512:(ih + 1) * 512], q_ps[:],
                              SQ / SM)

        # ---- P2T[j, o] = sum_c hn[c, j] W2T[c, o], DR fp8 ----
        for jc in range(JCN):
            p_ps = pearly.tile([P, 512], F32, name="p_ps", tag="big")
            for h in range(2):
                nc.tensor.matmul(p_ps[:],
                                 hn[:, 2 * h:2 * h + 2, jc * P:(jc + 1) * P],
                                 W2T8[:, 2 * h:2 * h + 2, :],
                                 start=(h == 0), stop=(h == 1),
                                 perf_mode=DR, skip_group_check=True)
            if jc % 2 == 0:
                nc.vector.tensor_scalar_mul(P2T8[:, jc, :], p_ps[:], SPP / SW2)
            else:
                nc.scalar.mul(P2T8[:, jc, :], p_ps[:], SPP / SW2)

        if with_t:
            # t[n] = sum_c' u[c'] hn[c', n] -> DRAM bounce -> t_part[p, jc]
            t_dram = dram.tile([N], F32)
            for nb in range(NB):
                t_ps = pearly.tile([1, 512], F32, name="t_ps", tag="big")
                for h in range(2):
                    nc.tensor.matmul(t_ps[:], ut8[:, 2 * h:2 * h + 2],
                                     hn[:, 2 * h:2 * h + 2,
                                        nb * 512:(nb + 1) * 512],
                                     start=(h == 0), stop=(h == 1),
                                     perf_mode=DR, skip_group_check=True)
                t_ch = evac.tile([1, 512], F32, name="t_ch", tag="tch", bufs=1)
                nc.scalar.mul(t_ch[:], t_ps[:], 1.0 / SU)
                nc.sync.dma_start(t_dram[nb * 512:(nb + 1) * 512], t_ch[:])
            nc.sync.dma_start(t_part[:], t_dram.rearrange("(jc p) -> p jc", p=P))

        pearly.release()

        # ---- attention ----
        # scores(ib0), scores(ib1) with exp -> fp8 aT pair tiles; then per
        # i-block: softmax denominator summed ON PE (ones8 DoubleRow against
        # each aT pair tile, accumulating a [1, 512] PSUM), the fused
        # (attention x out-proj) GEMM accumulating P2T^T a over all 32 j
        # chunks, and evacuation normalized by 1/(SPP*den).
        aTpool = tc.alloc_tile_pool(name="aT", bufs=34)
        patt = tc.alloc_tile_pool(name="patt", bufs=1, space="PSUM")
        aTs = {}
        for ib in range(IBN):
            i0 = ib * 512
            for jc in range(JCN):
                u, par = divmod(jc, 2)
                s_ps = patt.tile([P, 512], F32, name="s_ps", tag="s", bufs=3)
                for h in range(2):
                    nc.tensor.matmul(s_ps[:],
                                     hn[:, 2 * h:2 * h + 2, jc * P:(jc + 1) * P],
                                     q28[:, 2 * h:2 * h + 2, i0:i0 + 512],
                                     start=(h == 0), stop=(h == 1),
                                     perf_mode=DR)
                if par == 0:
                    aT_t = aTpool.tile([P, 2, 512], FP8, name="aT_t", tag="aT",
                                       bufs=34)
                    aTs[ib, u] = aT_t
                aT_t = aTs[ib, u]
                if with_t:
                    nc.scalar.activation(aT_t[:, par, :], s_ps[:], AF.Exp,
                                         bias=t_part[:, jc:jc + 1],
                                         scale=1.0 / SQ)
                else:
                    nc.scalar.activation(aT_t[:, par, :], s_ps[:], AF.Exp,
                                         scale=1.0 / SQ)

        for ib in range(IBN):
            # den[i] = sum_j a[j, i] on PE, broadcast to all 128 partitions
            # by the all-ones stationary; reciprocal on DVE overlaps the
            # AVproj matmuls that follow.
            den_ps = patt.tile([P, 512], F32, name=f"den_ps{ib}", tag="s",
                               bufs=3)
            for u in range(UCN):
                nc.tensor.matmul(den_ps[:], ones8[:], aTs[ib, u][:],
                                 start=(u == 0), stop=(u == UCN - 1),
                                 perf_mode=DR, skip_group_check=True)
            recip = const.tile([P, 512], BF16, name=f"recip{ib}")
            with nc.allow_low_precision(reason="bf16 1/denominator is ample"):
                nc.vector.reciprocal(recip[:], den_ps[:])

            # fused (attention x out-proj): op[o, i] = sum_j P2T[j, o] a[j, i]
            i0 = ib * 512
            for oc in range(CCN):
                op_ps = patt.tile([P, 512], F32, name=f"op_ps{oc}", tag="av",
                                  bufs=4)
                for u in range(UCN):
                    nc.tensor.matmul(op_ps[:],
                                     P2T8[:, 2 * u:2 * u + 2,
                                          oc * P:(oc + 1) * P],
                                     aTs[ib, u][:],
                                     start=(u == 0), stop=(u == UCN - 1),
                                     perf_mode=DR, skip_group_check=True)
                osb = evac.tile([P, 512], BF16, name="osb", tag="osb")
                nc.vector.scalar_tensor_tensor(osb[:], op_ps[:],
                                               1.0 / SPP, recip[:],
                                               op0=ALU.mult, op1=ALU.mult)
                xr = xfull[:, oc, i0:i0 + 512]
                if with_b2:
                    nc.vector.scalar_tensor_tensor(osb[:], osb[:],
                                                   b2_sb[:, oc:oc + 1], xr,
                                                   op0=ALU.add, op1=ALU.add)
                elif oc % 2 == 0:
                    nc.gpsimd.tensor_add(osb[:], osb[:], xr)
                else:
                    nc.vector.tensor_add(osb[:], osb[:], xr)
                nc.sync.dma_start(ov[:, oc, i0:i0 + 512], osb[:])

        aTpool.release()
        patt.release()
        xpool.release()


# ---------------- host side ----------------

_CACHED = {}


def _get_nc(with_t, with_b2):
    key = (with_t, with_b2)
    if key not in _CACHED:
        _CACHED[key] = build_program(with_t=with_t, with_b2=with_b2)
    return _CACHED[key]


def _host_constants():
    p = np.arange(P)
    sel = np.zeros((P, 8), np.float32)
    sel[p, p // GROUP] = 1.0 / GROUP
    bsel = np.zeros((8, P), np.float32)
    bsel[p // GROUP, p] = 1.0
    ones8 = np.ones((P, 2 * P), dtype=FP8NP)
    return dict(sel=sel, bsel=bsel, ones8=ones8)


def _host_weights(wq, bq, wk, wv, bv, wo, bo):
    """Weights-only folds (input-independent): M, W2, b2, u."""
    wq = np.asarray(wq, np.float32)
    wk = np.asarray(wk, np.float32)
    wv = np.asarray(wv, np.float32)
    wo = np.asarray(wo, np.float32)
    M = (wq.T @ wk) * SCALE
    W2 = wo @ wv
    b2 = wo @ np.asarray(bv, np.float32) + np.asarray(bo, np.float32)
    u = (wk.T @ np.asarray(bq, np.float32)) * SCALE
    return (np.ascontiguousarray((M * SM).astype(FP8NP)),
            np.ascontiguousarray((W2.T * SW2).astype(FP8NP)),
            b2.astype(np.float32),
            (u * SU).astype(FP8NP))


def kernel(x, gn_scale, gn_bias, wq, bq, wk, bk, wv, bv, wo, bo):
    from concourse.bass_utils import run_bass_kernel_spmd

    m8, w2t8, b2, u8 = _host_weights(wq, bq, wk, wv, bv, wo, bo)
    with_t = bool(np.any(np.asarray(bq, np.float32) != 0))
    with_b2 = bool(np.any(b2 != 0))
    nc = _get_nc(with_t, with_b2)
    consts = _host_constants()
    xr = np.ascontiguousarray(
        np.asarray(x, np.float32).reshape(2, C, N).astype(BF16NP))
    shared = dict(
        m8=m8, w2t8=w2t8, b2=b2, u8=u8,
        gamma=np.asarray(gn_scale, np.float32),
        beta=np.asarray(gn_bias, np.float32),
        **consts,
    )
    in_maps = []
    for core in range(8):
        b, qc = divmod(core, 4)
        perm_x = np.concatenate(
            [xr[b][:, qc * NQ:(qc + 1) * NQ],
             np.delete(xr[b], np.s_[qc * NQ:(qc + 1) * NQ], axis=1)], axis=1)
        in_maps.append({"xp": np.ascontiguousarray(perm_x), **shared})

    res = run_bass_kernel_spmd(nc, in_maps, core_ids=list(range(8)))
    y = np.empty((2, C, N), np.float32)
    for core in range(8):
        b, qc = divmod(core, 4)
        y[b][:, qc * NQ:(qc + 1) * NQ] = res.results[core]["out"].astype(
            np.float32)
    return y.reshape(2, C, 64, 64)
